# revision 55
# baseline (speedup 1.0000x reference)
"""Causal multi-head attention block on 8 TRN2 NeuronCores.

Sharding: tensor-parallel over heads (2 heads/core, both batches) for the
QKV projection + attention; an on-device AllToAll re-shards to
sequence-parallel for the output projection (Megatron-style).

v2 structure: QKV projection and attention are fused into one software
pipeline over (batch, q-chunk) — projection of chunk c+1 is emitted
interleaved with attention of chunk c so PE and ACT overlap instead of
running as serial phases.  Init DMAs are batched (one DMA per x chunk via a
4-dim AP, one strided DMA for the VA ones block).  The V projection bias is
folded into the output bias on the host (softmax rows sum to 1), so V needs
no on-device bias add.  Diagonal score tiles truncate the matmul, exp, and
mask to the causal region.

Matmuls run in float32r (full PE rate at N>=256, ~1.5e-4 rel err).  Scores
use zero-padded per-head K^T copies so they run as full K=128 matmuls
(half-height K=64 row-tiled matmuls measured 1.8x slower per-op on HW).

Self-contained: hardcodes all shapes from the problem spec.
"""

import numpy as np
from contextlib import ExitStack

import concourse.bass as bass
import concourse.tile as tile
from concourse import bacc, mybir
from concourse.bass_utils import run_bass_kernel_spmd

F32R = mybir.dt.float32r
F32 = mybir.dt.float32
BF16 = mybir.dt.bfloat16
AF = mybir.ActivationFunctionType

B, T, C, H, HD = 2, 2048, 1024, 16, 64
NCORES = 8
BT = B * T            # 4096 global rows
TQ = 512              # q-chunk width
KT = 128              # k-tile height
NJ = T // TQ          # 4 q-chunks per batch (= per core)
NKK = T // KT         # 16 k-tiles per batch
NCT = C // 128        # 8 contraction tiles for projections
NTC = BT // TQ        # 8 global t-chunks
TSL = BT // NCORES    # 512 rows of final output per core
# chunk-contiguous full x^T: xt[tc0, p, kc, q] — each chunk's load is one
# fully-contiguous DMA (strided HBM reads measured ~8x below spec BW)
XT_SHAPE = [NTC, 128, NCT, TQ]
# pipeline emission style: "hybrid" = first k-tiles of attend(c) emitted
# before the interleaved projection block (keeps ACT fed), "chunk" =
# projections emitted whole before each attend, "phase" = all projections
# then all attends.
INTERLEAVE = "chunk"
V_CONSOL = False      # V quarters share one PSUM tile + 2 big VA copies
ORDER_ROT = False     # rotate batch-1 attends so the tail chunk is short
XPOOL_BUFS = 2
PLAN_EARLY = True     # attend(0,0) starts right after proj(0)
SEL1 = False          # one affine_select covers both heads' mask regions
PSUM_SHARE = False    # projection PSUM shares the score pool (3 slots)
# bf16 input path: x and w_qkv shipped/loaded as bf16 (halves the dominant
# x HBM traffic; QKV projection matmuls run bf16 at the same PE rate).
XT_BF16 = True
# bf16 attention output: y, A2A payload, and w_out in bf16 (halves the
# collective payload and the phase-4 weight/activation traffic).
Y_BF16 = True


def declare_io(nc):
    """DRAM tensors shared by build() and the timing loop builder."""
    xdt = BF16 if XT_BF16 else F32R
    ydt = BF16 if Y_BF16 else F32R
    d = {}
    d["xt"] = nc.dram_tensor("xt", XT_SHAPE, xdt, kind="ExternalInput")
    d["wqkv"] = nc.dram_tensor("wqkv", [128, 3, NCT, 128], xdt,
                               kind="ExternalInput")
    d["bqkv"] = nc.dram_tensor("bqkv", [128, 3], F32, kind="ExternalInput")
    d["wout"] = nc.dram_tensor("wout", [128, NCT, C], ydt,
                               kind="ExternalInput")
    d["bout"] = nc.dram_tensor("bout", [128, C], F32, kind="ExternalInput")
    d["out"] = nc.dram_tensor("out", [TSL, C], F32, kind="ExternalOutput")
    d["vaones"] = nc.dram_tensor("vaones", [128, 2, NKK, 64], F32R,
                                 kind="ExternalInput")
    d["zeros"] = nc.dram_tensor("zeros", [64, T], F32R, kind="ExternalInput")
    return d


def build(with_collective=True):
    nc = bacc.Bacc(None, target_bir_lowering=False)
    d = declare_io(nc)
    ydt = BF16 if Y_BF16 else F32R
    a2a_in = nc.dram_tensor("a2a_in", [NCORES, 128, TQ], ydt)
    if with_collective is None:
        a2a_out = a2a_in
    else:
        a2a_out = nc.dram_tensor("a2a_out", [NCORES, 128, TQ], ydt)
    with tile.TileContext(nc) as tc:
        _emit(nc, tc, d, a2a_in, a2a_out, with_collective)
    nc.compile()
    return nc


def _emit(nc, tc, d, a2a_in, a2a_out, with_collective, trunc=None):
    xt, wqkv, bqkv = d["xt"], d["wqkv"], d["bqkv"]
    wout, bout, out = d["wout"], d["bout"], d["out"]

    with ExitStack() as ctx:
        persist = ctx.enter_context(tc.tile_pool(name="persist", bufs=1))

        # persistent SBUF tensors, indexed by batch b (the core owns the
        # same 2 heads in both batches).
        qts = [persist.tile([128, T], F32R, tag=f"qt{p}", name=f"qt{p}")
               for p in range(2)]
        # zero-padded per-head K^T (head h lives in rows 64*(h%2);
        # the other 64 rows are zero so scores run as full K=128 matmuls)
        kts = [persist.tile([128, T], F32R, tag=f"kt{h}", name=f"kt{h}")
               for h in range(4)]
        xdt = BF16 if XT_BF16 else F32R
        ydt = BF16 if Y_BF16 else F32R
        va = persist.tile([128, 2, NKK, 192], F32R, tag="va")  # [V_e|ones|V_o]
        wsb = persist.tile([128, 3, NCT, 128], xdt, tag="wsb")
        bsb = persist.tile([128, 3], F32, tag="bsb")
        wosb = persist.tile([128, NCT, C], ydt, tag="wo")
        bosb = persist.tile([128, C], F32, tag="bo")

        # per-group weight loads so the Q slice lands first
        for g3 in range(3):
            nc.sync.dma_start(wsb[:, g3], wqkv[:, g3])
        nc.sync.dma_start(bsb[:], bqkv[:])

        def init_rest():
            # deferred so the first x-chunk DMA isn't queued behind these
            # zero the dead half of each per-head K^T (one DMA per head)
            for h in range(4):
                dead = slice(64, 128) if h % 2 == 0 else slice(0, 64)
                nc.sync.dma_start(kts[h][dead, :], d["zeros"][:])
            # ones block of VA in one strided DMA
            nc.sync.dma_start(va[:, :, :, 64:128], d["vaones"][:])

        pipe = ctx.enter_context(ExitStack())
        pools = {}
        pools["x"] = pipe.enter_context(
            tc.tile_pool(name="xtile", bufs=XPOOL_BUFS))
        if PSUM_SHARE:
            pools["s"] = pipe.enter_context(
                tc.tile_pool(name="psc", bufs=3, space="PSUM"))
            pools["pp"] = pools["s"]
        else:
            pools["pp"] = pipe.enter_context(
                tc.tile_pool(name="pp", bufs=2, space="PSUM"))
            pools["s"] = pipe.enter_context(
                tc.tile_pool(name="psc", bufs=2, space="PSUM"))
        pools["o"] = pipe.enter_context(
            tc.tile_pool(name="po", bufs=2, space="PSUM"))
        pools["pt"] = pipe.enter_context(tc.tile_pool(name="ptp", bufs=5))
        pools["yt"] = pipe.enter_context(tc.tile_pool(name="yt", bufs=2))
        pools["rt"] = pipe.enter_context(tc.tile_pool(name="rt", bufs=2))
        pools["oe"] = pipe.enter_context(tc.tile_pool(name="oe", bufs=2))

        def proj_steps(tc0):
            """Generator: emit projection of chunk tc0, yielding between
            units so attention of the previous chunk can interleave."""
            b, jloc = divmod(tc0, NJ)
            chunk = slice(TQ * jloc, TQ * (jloc + 1))
            xtile = pools["x"].tile([128, NCT, TQ], xdt, tag="x",
                                    name=f"x{tc0}")
            nc.sync.dma_start(xtile[:], xt[tc0])
            yield
            ptag = "s" if PSUM_SHARE else "pp"
            for g in range(2):          # 0 = Q^T, 1 = K^T
                ps = pools["pp"].tile([128, TQ], F32, tag=ptag,
                                      name=f"pp{tc0}_{g}")
                for kc in range(NCT):
                    nc.tensor.matmul(ps[:], wsb[:, g, kc], xtile[:, kc, :],
                                     start=(kc == 0), stop=(kc == NCT - 1))
                    if kc % 2 == 1:
                        yield
                if g == 0:
                    nc.vector.tensor_scalar_add(qts[b][:, chunk], ps[:],
                                                bsb[:, 0:1])
                else:
                    nc.vector.tensor_scalar_add(
                        kts[2 * b][0:64, chunk], ps[0:64, :], bsb[0:64, 1:2])
                    nc.vector.tensor_scalar_add(
                        kts[2 * b + 1][64:128, chunk], ps[64:128, :],
                        bsb[64:128, 1:2])
                yield
            # V directly in [keys, dims] layout (x^T slice as the stationary
            # operand); V bias is folded into the output bias on the host.
            if V_CONSOL:
                psv = pools["pp"].tile([128, 4, 128], F32, tag=ptag,
                                       name=f"ppv{tc0}")
                for q in range(4):
                    for kc in range(NCT):
                        nc.tensor.matmul(psv[:, q],
                                         xtile[:, kc, 128 * q:128 * (q + 1)],
                                         wsb[:, 2, kc],
                                         start=(kc == 0),
                                         stop=(kc == NCT - 1))
                    if q % 2 == 1:
                        yield
                tt4 = slice(jloc * 4, jloc * 4 + 4)  # k-tiles in batch b
                nc.vector.tensor_copy(va[:, b, tt4, 0:64], psv[:, :, 0:64])
                nc.vector.tensor_copy(va[:, b, tt4, 128:192],
                                      psv[:, :, 64:128])
                yield
            else:
                for q in range(4):
                    tt = jloc * 4 + q   # k-tile index in batch b
                    psv = pools["pp"].tile([128, TQ], F32, tag=ptag,
                                           name=f"ppv{tc0}_{q}")
                    for kc in range(NCT):
                        nc.tensor.matmul(psv[:, 0:128],
                                         xtile[:, kc, 128 * q:128 * (q + 1)],
                                         wsb[:, 2, kc],
                                         start=(kc == 0),
                                         stop=(kc == NCT - 1))
                    nc.vector.tensor_copy(va[:, b, tt, 0:64], psv[:, 0:64])
                    nc.vector.tensor_copy(va[:, b, tt, 128:192],
                                          psv[:, 64:128])
                    yield

        def attend_steps(p, j):
            """Generator: emit attention for q-chunk (p, j), yielding after
            each k-tile."""
            nkk = 4 * (j + 1)
            po = [pools["o"].tile([128, TQ], F32, tag="po",
                                  name=f"po{p}_{j}_{h}") for h in range(2)]
            for kk in range(nkk):
                o = max(kk - 4 * j, 0)  # suffix offset (diagonal tiles)
                lo = KT * o
                ps_s = pools["s"].tile([128, 2, TQ], F32, tag="s",
                                       name=f"s{p}_{j}_{kk}")
                for h2 in range(2):
                    nc.tensor.matmul(
                        ps_s[:, h2, lo:],
                        kts[2 * p + h2][:, KT * kk:KT * (kk + 1)],
                        qts[p][:, TQ * j + lo:TQ * (j + 1)],
                        start=True, stop=True)
                pt = pools["pt"].tile([128, 2, TQ], F32R, tag="pt",
                                      name=f"p{p}_{j}_{kk}")
                nc.scalar.activation(pt[:, :, lo:], ps_s[:, :, lo:], AF.Exp)
                if kk >= 4 * j:
                    if SEL1:
                        # aligned triangle for both heads in one op: the
                        # head axis gets affine coefficient 0
                        nc.gpsimd.affine_select(
                            out=pt[:, :, lo:],
                            in_=pt[:, :, lo:],
                            compare_op=mybir.AluOpType.is_ge,
                            fill=0.0, base=0,
                            pattern=[[0, 2], [1, TQ - lo]],
                            channel_multiplier=-1)
                    else:
                        for h2 in range(2):
                            # aligned triangle: keep qf' >= r
                            nc.gpsimd.affine_select(
                                out=pt[:, h2, lo:],
                                in_=pt[:, h2, lo:],
                                compare_op=mybir.AluOpType.is_ge,
                                fill=0.0, base=0,
                                pattern=[[1, TQ - lo]],
                                channel_multiplier=-1)
                # yield here so filler PE work can cover the exp latency
                # between this k-tile's score and AV matmuls
                yield
                for h2 in range(2):
                    vs = slice(0, 128) if h2 == 0 else slice(64, 192)
                    nc.tensor.matmul(
                        po[h2][:, lo:], va[:, p, kk, vs], pt[:, h2, lo:],
                        start=(kk == 0), stop=(kk == nkk - 1))
                yield
            # normalize: h0 sums in rows 64:128, h1 sums in rows 0:64
            # (copy psum->sbuf fast so the accumulator banks free early)
            oes = [pools["oe"].tile([128, TQ], F32, tag="oe",
                                    name=f"oe{p}_{j}_{h}") for h in range(2)]
            nc.vector.tensor_copy(oes[0][:], po[0][:])
            nc.vector.tensor_copy(oes[1][:], po[1][:])
            yt = pools["yt"].tile([128, TQ], ydt, tag="yt", name=f"y{p}_{j}")
            rt = pools["rt"].tile([128, TQ], F32, tag="rt", name=f"r{p}_{j}")
            nc.vector.reciprocal(rt[0:64, :], oes[0][64:128, :])
            nc.vector.tensor_mul(yt[0:64, :], oes[0][0:64, :], rt[0:64, :])
            nc.vector.reciprocal(rt[64:128, :], oes[1][0:64, :])
            nc.vector.tensor_mul(yt[64:128, :], oes[1][64:128, :],
                                 rt[64:128, :])
            nc.sync.dma_start(a2a_in[p * NJ + j, :, :], yt[:])

        # ---- fused pipeline: proj(c+1) interleaved with attend(c) ----
        def drain(g):
            if g is not None:
                for _ in g:
                    pass

        # batch 1 optionally rotated so the last attend chunk is a short one
        # (4 k-tiles), shrinking the serial tail before the A2A.
        if ORDER_ROT:
            order = [(0, 0), (0, 1), (0, 2), (0, 3),
                     (1, 1), (1, 2), (1, 3), (1, 0)]
        else:
            order = [(p, j) for p in range(2) for j in range(NJ)]
        # projection emission plan: chunk 0 up front, attend(0,0) starts
        # immediately after it, then one projection block per attend step
        if PLAN_EARLY:
            proj_plan = {1: [1, 2], 2: [3], 3: [4], 4: [5], 5: [6], 6: [7]}
            upfront = [0]
        else:
            proj_plan = {i: [i + 2] for i in range(NTC - 2)}
            upfront = [0, 1]

        def start_proj(idx):
            g = proj_steps(idx)
            next(g)          # emits the x-chunk DMA
            if idx == 0:
                init_rest()
            return g

        if trunc == "proj" or INTERLEAVE == "phase":
            for idx in range(NTC):
                drain(start_proj(idx))
            if trunc != "proj":
                for idx, (p, j) in enumerate(order):
                    if idx == 4:
                        nc.sync.dma_start(wosb[:], wout[:])
                        nc.sync.dma_start(bosb[:], bout[:])
                    drain(attend_steps(p, j))
        else:
            for k in upfront:
                drain(start_proj(k))
            for idx, (p, j) in enumerate(order):
                if idx == 4:
                    nc.sync.dma_start(wosb[:], wout[:])
                    nc.sync.dma_start(bosb[:], bout[:])
                pgs = [start_proj(k) for k in proj_plan.get(idx, [])]
                ag = attend_steps(p, j)
                if INTERLEAVE == "fill" and pgs:
                    # pace projection units into the attend yield points --
                    # one sits between each k-tile's score and AV matmuls,
                    # covering the exp latency with PE work
                    from itertools import chain
                    pchain = chain(*pgs)
                    ny = 8 * (j + 1)
                    np_est = 15 * len(pgs)
                    acc = 0
                    for i, _ in enumerate(ag):
                        want = ((i + 1) * np_est) // ny
                        while pchain is not None and acc < want:
                            if next(pchain, "end") == "end":
                                pchain = None
                                break
                            acc += 1
                    drain(pchain)
                    continue
                if INTERLEAVE == "hybrid":
                    # prime ACT with the first k-tiles before the proj block
                    for _ in range(2):
                        next(ag, None)
                for pg in pgs:
                    drain(pg)
                drain(ag)

        pipe.close()

        if trunc in ("proj", "attn"):
            with tc.tile_pool(name="dumo", bufs=1) as dpool:
                dm = dpool.tile([128, TQ], F32, tag="d")
                nc.vector.tensor_copy(dm[:], qts[0][0:128, 0:TQ].bitcast(F32))
                nc.sync.dma_start(out[0:128, 0:TQ], dm[:])
            return

        # ---- all-to-all (head-sharded -> t-sharded) ----
        if with_collective is True:
            nc.gpsimd.collective_compute(
                "AllToAll", mybir.AluOpType.bypass,
                replica_groups=[list(range(NCORES))],
                ins=[a2a_in[:]], outs=[a2a_out[:]])
        elif with_collective is False:
            nc.sync.dma_start(a2a_out[:], a2a_in[:])
        # else (None): timing mode — caller aliases a2a_out to a2a_in

        # ---- output projection (rows TSL per core) ----
        with (
            tc.tile_pool(name="yts", bufs=1) as ytspool,
            tc.tile_pool(name="pout", bufs=4, space="PSUM") as poutp,
            tc.tile_pool(name="osb", bufs=4) as osbpool,
        ):
            yts = ytspool.tile([128, NCT, TQ], ydt, tag="yts")
            for cc in range(NCT):
                nc.sync.dma_start(yts[:, cc, :], a2a_out[cc, :, :])

            for tt in range(TSL // 128):
                pos = [poutp.tile([128, TQ], F32, tag="pout",
                                  name=f"pos{tt}_{h}") for h in range(2)]
                for cc in range(NCT):
                    for n in range(2):
                        nc.tensor.matmul(
                            pos[n][:], yts[:, cc, 128 * tt:128 * (tt + 1)],
                            wosb[:, cc, TQ * n:TQ * (n + 1)],
                            start=(cc == 0), stop=(cc == NCT - 1))
                for n in range(2):
                    osb = osbpool.tile([128, TQ], F32, tag="osb")
                    nc.vector.tensor_add(osb[:], pos[n][:],
                                         bosb[:, TQ * n:TQ * (n + 1)])
                    nc.sync.dma_start(
                        out[128 * tt:128 * (tt + 1), TQ * n:TQ * (n + 1)],
                        osb[:])


def make_core_inputs(x, w_qkv, b_qkv, w_out, b_out):
    """Host-side shard/transform. Returns list of per-core input dicts."""
    x = np.asarray(x, np.float32)
    w_qkv = np.asarray(w_qkv, np.float32)
    b_qkv = np.asarray(b_qkv, np.float32)
    w_out = np.asarray(w_out, np.float32)
    b_out = np.asarray(b_out, np.float32)

    import ml_dtypes
    xdt = ml_dtypes.bfloat16 if XT_BF16 else np.float32
    ydt = ml_dtypes.bfloat16 if Y_BF16 else np.float32

    # softmax rows sum to 1, so the V bias contributes (b_v @ w_out) to
    # every output row — fold it into the output bias.
    b_eff = b_out + b_qkv[2 * C:] @ w_out.astype(ydt).astype(np.float32)
    bout_rep = np.ascontiguousarray(
        np.broadcast_to(b_eff.astype(np.float32), (128, C)))
    # chunk-contiguous x^T: xt[tc0, p, kc, q] = x_flat[TQ*tc0+q, 128*kc+p]
    xt = np.ascontiguousarray(
        x.reshape(NTC, TQ, NCT, 128).transpose(0, 3, 2, 1)).astype(xdt)
    # weight layouts matching the on-device SBUF tiles (contiguous DMAs)
    wout2 = np.ascontiguousarray(
        w_out.reshape(NCT, 128, C).transpose(1, 0, 2)).astype(ydt)
    vaones = np.ones((128, 2, NKK, 64), np.float32)
    zeros = np.zeros((64, T), np.float32)
    in_maps = []
    for c in range(NCORES):
        s = slice(128 * c, 128 * (c + 1))
        wq = w_qkv[:, :C][:, s] * 0.125
        wk = w_qkv[:, C:2 * C][:, s]
        wv = w_qkv[:, 2 * C:][:, s]
        wc = np.concatenate([wq, wk, wv], axis=1)
        wc2 = np.ascontiguousarray(
            wc.reshape(NCT, 128, 3, 128).transpose(1, 2, 0, 3)).astype(xdt)
        bc3 = np.ascontiguousarray(
            np.stack([b_qkv[:C][s] * 0.125, b_qkv[C:2 * C][s],
                      np.zeros(128, np.float32)], axis=1))
        in_maps.append({
            "xt": xt, "wqkv": wc2, "bqkv": bc3,
            "wout": wout2, "bout": bout_rep,
            "vaones": vaones, "zeros": zeros,
        })
    return in_maps


_NC_CACHE = {}


def kernel(x, w_qkv, b_qkv, w_out, b_out):
    in_maps = make_core_inputs(x, w_qkv, b_qkv, w_out, b_out)
    if "nc" not in _NC_CACHE:
        _NC_CACHE["nc"] = build()
    nc = _NC_CACHE["nc"]
    res = run_bass_kernel_spmd(nc, in_maps, core_ids=list(range(NCORES)))
    full = np.concatenate([res.results[c]["out"] for c in range(NCORES)],
                          axis=0)
    return full.reshape(B, T, C)
